# revision 5
# baseline (speedup 1.0000x reference)
"""Multi-head attention (B=4, N=2048, DIM=768, H=8, DH=96) on 8 TRN2 NeuronCores.

Sharding: data-parallel over (batch, query-half) — core c handles batch c//2,
query rows [(c%2)*1024, (c%2+1)*1024). Each core computes K/V for its full
batch (duplicated across the 2 cores sharing a batch), so there are NO
collectives: each core produces its own disjoint output shard.

Per-core compute (all matmuls bf16, fp32 PSUM accumulation):
  - Q^T/K^T projection in transposed space with head-dim padded 96->128 so
    each head's Q^T/K^T lands in its own 128-partition tile.
    lhsT = w^T[c,128f] chunk, rhs = x^T[c, n] chunk.
  - V projection in natural space: lhsT = x^T[c, 128n] chunk, rhs = w_v[c, f].
    A constant 1.0 column is appended per head (V|1) so the attn@V matmul
    also produces the softmax row-sums.
  - dots: P^T[nk,nq] = (K^T chunk)^T^T... out = lhsT.T@rhs with
    lhsT=K^T[128dh, 128nk], rhs=Q^T[128dh, 512nq]; softmax scale is folded
    into w_q on the host. exp() on ScalarE PSUM->SBUF(bf16), no max
    subtraction (logits max ~9, fp32-safe).
  - O'^T[97, nq] += (V|1)[128nk, 97]^T.T @ P^T tile; row 96 is the row-sum s.
  - normalize: r = 1/s (DVE), broadcast r across partitions with a K=1
    outer-product matmul (ones[1,96]^T @ r[1,512] -> [96,512] PSUM), then
    O^T = O'^T * r_bcast (DVE) -> bf16.
  - proj: y^T[c,nq] = sum_h w_p^T[h][96, c-chunk]^T.T... accumulated over
    8 heads in PSUM; bias added during PSUM->SBUF eviction; DMA out.

Output per core: y^T [768, 1024] fp32; host transposes/reassembles.
"""

import numpy as np
import ml_dtypes

B, N, DIM = 4, 2048, 768
H, DH = 8, 96
DHP = 128          # padded head dim for Q/K tiles
NQ = N // 2        # query rows per core
SCALE = DH ** -0.5
NCORES = 8
CT = DIM // 128    # 6 contraction chunks
NT = N // 128      # 16 key tiles
NQC = NQ // 512    # 2 query chunks of 512

_CACHE = {}


def _build():
    import concourse.mybir as mybir
    import concourse.tile as tile
    from concourse import bacc

    f32 = mybir.dt.float32
    bf16 = mybir.dt.bfloat16
    Exp = mybir.ActivationFunctionType.Exp
    mult = mybir.AluOpType.mult

    nc = bacc.Bacc("TRN2", debug=False, num_devices=NCORES)

    xt_d = nc.dram_tensor("xt", [DIM, N], bf16, kind="ExternalInput")
    wq_d = nc.dram_tensor("wq", [DIM, H * DHP], bf16, kind="ExternalInput")
    wk_d = nc.dram_tensor("wk", [DIM, H * DHP], bf16, kind="ExternalInput")
    wv_d = nc.dram_tensor("wv", [DIM, DIM], bf16, kind="ExternalInput")
    wp_d = nc.dram_tensor("wp", [H, DH, DIM], bf16, kind="ExternalInput")
    bias_d = nc.dram_tensor("bias", [DIM, 1], f32, kind="ExternalInput")
    out_d = nc.dram_tensor("out", [DIM, NQ], f32, kind="ExternalOutput")

    with tile.TileContext(nc) as tc:
        with (
            tc.tile_pool(name="const", bufs=1) as cpool,
            tc.tile_pool(name="ptp", bufs=4) as pt_pool,
            tc.tile_pool(name="onp", bufs=1) as on_pool,
            tc.tile_pool(name="smallp", bufs=2) as small_pool,
            tc.tile_pool(name="ysb", bufs=2) as y_pool,
            tc.tile_pool(name="ps_qkv", bufs=2, space="PSUM") as psum_qkv,
            tc.tile_pool(name="ps_d", bufs=2, space="PSUM") as psum_d,
            tc.tile_pool(name="ps_o", bufs=2, space="PSUM") as psum_o,
        ):
            # ---- persistent SBUF tensors + input DMAs ----
            xt_sb = [cpool.tile([128, N], bf16, name=f"xt{t}") for t in range(CT)]
            wq_sb = [cpool.tile([128, H * DHP], bf16, name=f"wq{t}") for t in range(CT)]
            wk_sb = [cpool.tile([128, H * DHP], bf16, name=f"wk{t}") for t in range(CT)]
            wv_sb = [cpool.tile([128, DIM], bf16, name=f"wv{t}") for t in range(CT)]
            wp_sb = [cpool.tile([DH, DIM], bf16, name=f"wp{h}") for h in range(H)]
            bias_sb = [cpool.tile([128, 1], f32, name=f"bias{t}") for t in range(CT)]
            qt_sb = [cpool.tile([DHP, NQ], bf16, name=f"qt{h}") for h in range(H)]
            kt_sb = [cpool.tile([DHP, N], bf16, name=f"kt{h}") for h in range(H)]
            v_sb = [cpool.tile([128, H, DH + 1], bf16, name=f"v{t}") for t in range(NT)]
            ones_sb = cpool.tile([128, DH], bf16, name="ones")

            for t in range(CT):
                nc.sync.dma_start(xt_sb[t][:], xt_d.ap()[t * 128:(t + 1) * 128, :])
            for t in range(CT):
                nc.sync.dma_start(wk_sb[t][:], wk_d.ap()[t * 128:(t + 1) * 128, :])
                nc.sync.dma_start(wq_sb[t][:], wq_d.ap()[t * 128:(t + 1) * 128, :])
                nc.sync.dma_start(wv_sb[t][:], wv_d.ap()[t * 128:(t + 1) * 128, :])
                nc.sync.dma_start(bias_sb[t][:], bias_d.ap()[t * 128:(t + 1) * 128, :])
            for h in range(H):
                nc.sync.dma_start(wp_sb[h][:], wp_d.ap()[h])

            nc.vector.memset(ones_sb[:], 1.0)
            for t in range(NT):
                nc.vector.memset(v_sb[t][:, :, DH:DH + 1], 1.0)

            # ---- V projection (natural orientation), 4 heads per pass ----
            def v_group(fc):
                for t in range(NT):
                    ps = psum_qkv.tile([128, 512], f32, name="vps", tag="qkvps")
                    vps = ps[:, :4 * DH]
                    for ct in range(CT):
                        nc.tensor.matmul(
                            vps,
                            lhsT=xt_sb[ct][:, t * 128:(t + 1) * 128],
                            rhs=wv_sb[ct][:, fc * 4 * DH:(fc + 1) * 4 * DH],
                            start=(ct == 0),
                            stop=(ct == CT - 1),
                        )
                    for j in range(4):
                        nc.vector.tensor_copy(
                            out=v_sb[t][:, fc * 4 + j, 0:DH],
                            in_=ps[:, j * DH:(j + 1) * DH],
                        )

            # ---- Q/K projection for one head (transposed orientation) ----
            def qkv_head(h):
                for nchunk in range(N // 512):
                    ps = psum_qkv.tile([128, 512], f32, name="kps", tag="qkvps")
                    for ct in range(CT):
                        nc.tensor.matmul(
                            ps,
                            lhsT=wk_sb[ct][:, h * DHP:(h + 1) * DHP],
                            rhs=xt_sb[ct][:, nchunk * 512:(nchunk + 1) * 512],
                            start=(ct == 0),
                            stop=(ct == CT - 1),
                        )
                    nc.vector.tensor_copy(
                        out=kt_sb[h][:, nchunk * 512:(nchunk + 1) * 512], in_=ps[:]
                    )
                for nchunk in range(NQ // 512):
                    ps = psum_qkv.tile([128, 512], f32, name="qps", tag="qkvps")
                    for ct in range(CT):
                        nc.tensor.matmul(
                            ps,
                            lhsT=wq_sb[ct][:, h * DHP:(h + 1) * DHP],
                            rhs=xt_sb[ct][:, nchunk * 512:(nchunk + 1) * 512],
                            start=(ct == 0),
                            stop=(ct == CT - 1),
                        )
                    nc.vector.tensor_copy(
                        out=qt_sb[h][:, nchunk * 512:(nchunk + 1) * 512], in_=ps[:]
                    )

            on_sb = [
                [on_pool.tile([DH, 512], bf16, name=f"on{h}_{qc}") for qc in range(NQC)]
                for h in range(H)
            ]

            # ---- attention for one head ----
            def attn_head(h):
                o_ps = [
                    psum_o.tile([DH + 1, 512], f32, name=f"ops{qc}", tag="ops")
                    for qc in range(NQC)
                ]
                for t in range(NT):
                    d_ps = psum_d.tile([128, NQC * 512], f32, name="dps", tag="dps")
                    for qc in range(NQC):
                        nc.tensor.matmul(
                            d_ps[:, qc * 512:(qc + 1) * 512],
                            lhsT=kt_sb[h][:, t * 128:(t + 1) * 128],
                            rhs=qt_sb[h][:, qc * 512:(qc + 1) * 512],
                            start=True,
                            stop=True,
                        )
                    pt = pt_pool.tile([128, NQC * 512], bf16, name="pt", tag="pt")
                    nc.scalar.activation(pt[:], d_ps[:], Exp)
                    for qc in range(NQC):
                        nc.tensor.matmul(
                            o_ps[qc],
                            lhsT=v_sb[t][:, h, :],
                            rhs=pt[:, qc * 512:(qc + 1) * 512],
                            start=(t == 0),
                            stop=(t == NT - 1),
                        )
                # Evacuate O' to SBUF immediately (frees the PSUM bank for the
                # next head); the slow normalize chain then runs off-path.
                for qc in range(NQC):
                    o_st = small_pool.tile(
                        [DH + 1, 512], f32, name="ostage", tag="ostage", bufs=3
                    )
                    nc.vector.tensor_copy(out=o_st[:], in_=o_ps[qc][:])
                    r = small_pool.tile([DH + 1, 512], bf16, name="recip", tag="recip")
                    with nc.allow_low_precision(reason="softmax normalizer in bf16"):
                        nc.vector.reciprocal(r[DH:DH + 1, :], o_st[DH:DH + 1, :])
                    r0 = small_pool.tile([1, 512], bf16, name="recip0", tag="recip0")
                    nc.sync.dma_start(r0[:], r[DH:DH + 1, :])
                    rb = psum_qkv.tile([128, 512], f32, name="rb", tag="qkvps")
                    nc.tensor.matmul(
                        rb[0:DH, :],
                        lhsT=ones_sb[0:1, :],
                        rhs=r0[:],
                        start=True,
                        stop=True,
                    )
                    nc.vector.tensor_tensor(
                        on_sb[h][qc][:], o_st[0:DH, :], rb[0:DH, :], mult
                    )

            # ---- emission order (for engine pipelining) ----
            v_group(0)
            qkv_head(0)
            attn_head(0)
            qkv_head(1)
            v_group(1)
            attn_head(1)
            for h in range(2, H):
                qkv_head(h)
                attn_head(h)

            # ---- output projection: y^T[c,nq] = sum_h wp[h]^T.T @ on[h] ----
            for ct in range(CT):
                y_sb = y_pool.tile([128, NQ], f32, name="y", tag="y")
                for qc in range(NQC):
                    yp = psum_qkv.tile([128, 512], f32, name="yps", tag="qkvps")
                    for h in range(H):
                        nc.tensor.matmul(
                            yp,
                            lhsT=wp_sb[h][:, ct * 128:(ct + 1) * 128],
                            rhs=on_sb[h][qc][:],
                            start=(h == 0),
                            stop=(h == H - 1),
                        )
                    nc.vector.tensor_scalar_add(
                        y_sb[:, qc * 512:(qc + 1) * 512], yp[:], bias_sb[ct][:]
                    )
                nc.sync.dma_start(out_d.ap()[ct * 128:(ct + 1) * 128, :], y_sb[:])

    nc.compile()
    return nc


def _get_nc():
    if "nc" not in _CACHE:
        _CACHE["nc"] = _build()
    return _CACHE["nc"]


def _prep_shards(x, w_qkv, w_proj, b_proj):
    bf16 = ml_dtypes.bfloat16
    x = np.asarray(x, dtype=np.float32)
    w_qkv = np.asarray(w_qkv, dtype=np.float32)
    w_proj = np.asarray(w_proj, dtype=np.float32)
    b_proj = np.asarray(b_proj, dtype=np.float32)

    # w_qkv: [3*INNER, DIM] rows: q rows [h*96+d], k rows 768+..., v rows 1536+...
    wqT = w_qkv[0:DIM].T.reshape(DIM, H, DH)        # [c, h, d]
    wkT = w_qkv[DIM:2 * DIM].T.reshape(DIM, H, DH)
    wvT = w_qkv[2 * DIM:3 * DIM].T                  # [c, f] natural head-major
    wq_pad = np.zeros((DIM, H, DHP), np.float32)
    wk_pad = np.zeros((DIM, H, DHP), np.float32)
    wq_pad[:, :, :DH] = wqT * SCALE
    wk_pad[:, :, :DH] = wkT
    wq_b = np.ascontiguousarray(wq_pad.reshape(DIM, H * DHP)).astype(bf16)
    wk_b = np.ascontiguousarray(wk_pad.reshape(DIM, H * DHP)).astype(bf16)
    wv_b = np.ascontiguousarray(wvT).astype(bf16)
    wp_b = np.ascontiguousarray(w_proj.T.reshape(H, DH, DIM)).astype(bf16)
    bias = np.ascontiguousarray(b_proj.reshape(DIM, 1))

    in_maps = []
    for c in range(NCORES):
        b, half = divmod(c, 2)
        xt = x[b].T  # [768, 2048]
        if half == 1:
            xt = np.concatenate([xt[:, NQ:], xt[:, :NQ]], axis=1)
        in_maps.append({
            "xt": np.ascontiguousarray(xt).astype(bf16),
            "wq": wq_b,
            "wk": wk_b,
            "wv": wv_b,
            "wp": wp_b,
            "bias": bias,
        })
    return in_maps


def kernel(x, w_qkv, w_proj, b_proj):
    from concourse.bass_utils import run_bass_kernel_spmd

    nc = _get_nc()
    in_maps = _prep_shards(x, w_qkv, w_proj, b_proj)
    res = run_bass_kernel_spmd(nc, in_maps, core_ids=list(range(NCORES)))
    out = np.empty((B, N, DIM), np.float32)
    for c in range(NCORES):
        b, half = divmod(c, 2)
        yT = np.asarray(res.results[c]["out"], dtype=np.float32)  # [768, 1024]
        out[b, half * NQ:(half + 1) * NQ, :] = yT.T
    return out


# revision 9
# speedup vs baseline: 1.1599x; 1.1599x over previous
"""Multi-head attention (B=4, N=2048, DIM=768, H=8, DH=96) on 8 TRN2 NeuronCores.

Sharding: data-parallel over (batch, query-half) — core c handles batch c//2,
query rows [(c%2)*1024, (c%2+1)*1024). Each core computes K/V for its full
batch (duplicated across the 2 cores sharing a batch), so there are NO
collectives: each core produces its own disjoint output shard.

Per-core compute (all matmuls bf16, fp32 PSUM accumulation):
  - Q^T/K^T projection in transposed space with head-dim padded 96->128 so
    each head's Q^T/K^T lands in its own 128-partition tile.
    lhsT = w^T[c,128f] chunk, rhs = x^T[c, n] chunk.
  - V projection in natural space: lhsT = x^T[c, 128n] chunk, rhs = w_v[c, f].
    A constant 1.0 column is appended per head (V|1) so the attn@V matmul
    also produces the softmax row-sums.
  - dots: P^T[nk,nq] = (K^T chunk)^T^T... out = lhsT.T@rhs with
    lhsT=K^T[128dh, 128nk], rhs=Q^T[128dh, 512nq]; softmax scale is folded
    into w_q on the host. exp() on ScalarE PSUM->SBUF(bf16), no max
    subtraction (logits max ~9, fp32-safe).
  - O'^T[97, nq] += (V|1)[128nk, 97]^T.T @ P^T tile; row 96 is the row-sum s.
  - normalize: r = 1/s (DVE), broadcast r across partitions with a K=1
    outer-product matmul (ones[1,96]^T @ r[1,512] -> [96,512] PSUM), then
    O^T = O'^T * r_bcast (DVE) -> bf16.
  - proj: y^T[c,nq] = sum_h w_p^T[h][96, c-chunk]^T.T... accumulated over
    8 heads in PSUM; bias added during PSUM->SBUF eviction; DMA out.

Output per core: y^T [768, 1024] fp32; host transposes/reassembles.
"""

import numpy as np
import ml_dtypes

B, N, DIM = 4, 2048, 768
H, DH = 8, 96
DHP = 128          # padded head dim for Q/K tiles
NQ = N // 2        # query rows per core
SCALE = DH ** -0.5
NCORES = 8
CT = DIM // 128    # 6 contraction chunks
NT = N // 128      # 16 key tiles
NQC = NQ // 512    # 2 query chunks of 512

_CACHE = {}


def _build():
    import concourse.mybir as mybir
    import concourse.tile as tile
    from concourse import bacc

    f32 = mybir.dt.float32
    bf16 = mybir.dt.bfloat16
    Exp = mybir.ActivationFunctionType.Exp
    mult = mybir.AluOpType.mult

    nc = bacc.Bacc("TRN2", debug=False, num_devices=NCORES)

    xt_d = nc.dram_tensor("xt", [DIM, N], bf16, kind="ExternalInput")
    wq_d = nc.dram_tensor("wq", [DIM, H * DHP], bf16, kind="ExternalInput")
    wk_d = nc.dram_tensor("wk", [DIM, H * DHP], bf16, kind="ExternalInput")
    wv_d = nc.dram_tensor("wv", [DIM, DIM], bf16, kind="ExternalInput")
    wp_d = nc.dram_tensor("wp", [H, DH, DIM], bf16, kind="ExternalInput")
    bias_d = nc.dram_tensor("bias", [DIM, 1], f32, kind="ExternalInput")
    out_d = nc.dram_tensor("out", [DIM, NQ], f32, kind="ExternalOutput")

    with tile.TileContext(nc) as tc:
        with (
            tc.tile_pool(name="const", bufs=1) as cpool,
            tc.tile_pool(name="ptp", bufs=4) as pt_pool,
            tc.tile_pool(name="onp", bufs=1) as on_pool,
            tc.tile_pool(name="smallp", bufs=2) as small_pool,
            tc.tile_pool(name="ysb", bufs=2) as y_pool,
            tc.tile_pool(name="ps_qkv", bufs=2, space="PSUM") as psum_qkv,
            tc.tile_pool(name="ps_d", bufs=3, space="PSUM") as psum_d,
            tc.tile_pool(name="ps_o", bufs=2, space="PSUM") as psum_o,
            tc.tile_pool(name="ps_rb", bufs=1, space="PSUM") as psum_rb,
        ):
            # ---- persistent SBUF tensors + input DMAs ----
            xt_sb = [cpool.tile([128, N], bf16, name=f"xt{t}") for t in range(CT)]
            wq_sb = [cpool.tile([128, H * DHP], bf16, name=f"wq{t}") for t in range(CT)]
            wk_sb = [cpool.tile([128, H * DHP], bf16, name=f"wk{t}") for t in range(CT)]
            wv_sb = [cpool.tile([128, DIM], bf16, name=f"wv{t}") for t in range(CT)]
            wp_sb = [cpool.tile([DH, DIM], bf16, name=f"wp{h}") for h in range(H)]
            bias_sb = [cpool.tile([128, 1], f32, name=f"bias{t}") for t in range(CT)]
            qt_sb = [cpool.tile([DHP, NQ], bf16, name=f"qt{h}") for h in range(H)]
            kt_sb = [cpool.tile([DHP, N], bf16, name=f"kt{h}") for h in range(H)]
            v_sb = [cpool.tile([128, H, DH + 1], bf16, name=f"v{t}") for t in range(NT)]
            ones_sb = cpool.tile([128, DH], bf16, name="ones")

            for t in range(CT):
                nc.sync.dma_start(xt_sb[t][:], xt_d.ap()[t * 128:(t + 1) * 128, :])
            for t in range(CT):
                nc.sync.dma_start(wk_sb[t][:], wk_d.ap()[t * 128:(t + 1) * 128, :])
                nc.sync.dma_start(wq_sb[t][:], wq_d.ap()[t * 128:(t + 1) * 128, :])
                nc.sync.dma_start(wv_sb[t][:], wv_d.ap()[t * 128:(t + 1) * 128, :])
                nc.sync.dma_start(bias_sb[t][:], bias_d.ap()[t * 128:(t + 1) * 128, :])
            for h in range(H):
                nc.sync.dma_start(wp_sb[h][:], wp_d.ap()[h])

            nc.vector.memset(ones_sb[:], 1.0)
            for t in range(NT):
                nc.vector.memset(v_sb[t][:, :, DH:DH + 1], 1.0)

            # ---- V projection (natural orientation), 4 heads per pass ----
            def v_group(fc):
                for t in range(NT):
                    ps = psum_qkv.tile([128, 512], f32, name="vps", tag="qkvps")
                    vps = ps[:, :4 * DH]
                    for ct in range(CT):
                        nc.tensor.matmul(
                            vps,
                            lhsT=xt_sb[ct][:, t * 128:(t + 1) * 128],
                            rhs=wv_sb[ct][:, fc * 4 * DH:(fc + 1) * 4 * DH],
                            start=(ct == 0),
                            stop=(ct == CT - 1),
                        )
                    for j in range(4):
                        nc.vector.tensor_copy(
                            out=v_sb[t][:, fc * 4 + j, 0:DH],
                            in_=ps[:, j * DH:(j + 1) * DH],
                        )

            # ---- Q/K projection for one head (transposed orientation) ----
            def qkv_head(h):
                for nchunk in range(N // 512):
                    ps = psum_qkv.tile([128, 512], f32, name="kps", tag="qkvps")
                    for ct in range(CT):
                        nc.tensor.matmul(
                            ps,
                            lhsT=wk_sb[ct][:, h * DHP:(h + 1) * DHP],
                            rhs=xt_sb[ct][:, nchunk * 512:(nchunk + 1) * 512],
                            start=(ct == 0),
                            stop=(ct == CT - 1),
                        )
                    nc.vector.tensor_copy(
                        out=kt_sb[h][:, nchunk * 512:(nchunk + 1) * 512], in_=ps[:]
                    )
                for nchunk in range(NQ // 512):
                    ps = psum_qkv.tile([128, 512], f32, name="qps", tag="qkvps")
                    for ct in range(CT):
                        nc.tensor.matmul(
                            ps,
                            lhsT=wq_sb[ct][:, h * DHP:(h + 1) * DHP],
                            rhs=xt_sb[ct][:, nchunk * 512:(nchunk + 1) * 512],
                            start=(ct == 0),
                            stop=(ct == CT - 1),
                        )
                    nc.vector.tensor_copy(
                        out=qt_sb[h][:, nchunk * 512:(nchunk + 1) * 512], in_=ps[:]
                    )

            on_sb = [
                [on_pool.tile([DH, 512], bf16, name=f"on{h}_{qc}") for qc in range(NQC)]
                for h in range(H)
            ]

            # ---- attention for one head ----
            def attn_head(h):
                o_ps = [
                    psum_o.tile([DH + 1, 512], f32, name=f"ops{qc}", tag="ops")
                    for qc in range(NQC)
                ]
                for t in range(NT):
                    for qc in range(NQC):
                        d_ps = psum_d.tile([128, 512], f32, name="dps", tag="dps")
                        nc.tensor.matmul(
                            d_ps[:],
                            lhsT=kt_sb[h][:, t * 128:(t + 1) * 128],
                            rhs=qt_sb[h][:, qc * 512:(qc + 1) * 512],
                            start=True,
                            stop=True,
                        )
                        pt = pt_pool.tile([128, 512], bf16, name="pt", tag="pt")
                        nc.scalar.activation(pt[:], d_ps[:], Exp)
                        nc.tensor.matmul(
                            o_ps[qc],
                            lhsT=v_sb[t][:, h, :],
                            rhs=pt[:],
                            start=(t == 0),
                            stop=(t == NT - 1),
                        )
                # Evacuate O' to SBUF immediately (frees the PSUM bank for the
                # next head); the slow normalize chain then runs off-path.
                for qc in range(NQC):
                    o_st = small_pool.tile(
                        [DH + 1, 512], f32, name="ostage", tag="ostage", bufs=3
                    )
                    nc.vector.tensor_copy(out=o_st[:], in_=o_ps[qc][:])
                    r = small_pool.tile([DH + 1, 512], bf16, name="recip", tag="recip")
                    with nc.allow_low_precision(reason="softmax normalizer in bf16"):
                        nc.vector.reciprocal(r[DH:DH + 1, :], o_st[DH:DH + 1, :])
                    r0 = small_pool.tile([1, 512], bf16, name="recip0", tag="recip0")
                    nc.sync.dma_start(r0[:], r[DH:DH + 1, :])
                    rb = psum_rb.tile([DH, 512], f32, name="rb", tag="rb")
                    nc.tensor.matmul(
                        rb[:],
                        lhsT=ones_sb[0:1, :],
                        rhs=r0[:],
                        start=True,
                        stop=True,
                    )
                    nc.vector.tensor_tensor(
                        on_sb[h][qc][:], o_st[0:DH, :], rb[:], mult
                    )

            # ---- emission order: qkv pipelined one head ahead of attention ----
            v_group(0)
            qkv_head(0)
            qkv_head(1)
            attn_head(0)
            qkv_head(2)
            v_group(1)
            attn_head(1)
            for h in range(2, H):
                if h + 1 < H:
                    qkv_head(h + 1)
                attn_head(h)

            # ---- output projection: y^T[c,nq] = sum_h wp[h]^T.T @ on[h] ----
            for ct in range(CT):
                y_sb = y_pool.tile([128, NQ], f32, name="y", tag="y")
                for qc in range(NQC):
                    yp = psum_qkv.tile([128, 512], f32, name="yps", tag="qkvps")
                    for h in range(H):
                        nc.tensor.matmul(
                            yp,
                            lhsT=wp_sb[h][:, ct * 128:(ct + 1) * 128],
                            rhs=on_sb[h][qc][:],
                            start=(h == 0),
                            stop=(h == H - 1),
                        )
                    nc.vector.tensor_scalar_add(
                        y_sb[:, qc * 512:(qc + 1) * 512], yp[:], bias_sb[ct][:]
                    )
                nc.sync.dma_start(out_d.ap()[ct * 128:(ct + 1) * 128, :], y_sb[:])

    nc.compile()
    return nc


def _get_nc():
    if "nc" not in _CACHE:
        _CACHE["nc"] = _build()
    return _CACHE["nc"]


def _prep_shards(x, w_qkv, w_proj, b_proj):
    bf16 = ml_dtypes.bfloat16
    x = np.asarray(x, dtype=np.float32)
    w_qkv = np.asarray(w_qkv, dtype=np.float32)
    w_proj = np.asarray(w_proj, dtype=np.float32)
    b_proj = np.asarray(b_proj, dtype=np.float32)

    # w_qkv: [3*INNER, DIM] rows: q rows [h*96+d], k rows 768+..., v rows 1536+...
    wqT = w_qkv[0:DIM].T.reshape(DIM, H, DH)        # [c, h, d]
    wkT = w_qkv[DIM:2 * DIM].T.reshape(DIM, H, DH)
    wvT = w_qkv[2 * DIM:3 * DIM].T                  # [c, f] natural head-major
    wq_pad = np.zeros((DIM, H, DHP), np.float32)
    wk_pad = np.zeros((DIM, H, DHP), np.float32)
    wq_pad[:, :, :DH] = wqT * SCALE
    wk_pad[:, :, :DH] = wkT
    wq_b = np.ascontiguousarray(wq_pad.reshape(DIM, H * DHP)).astype(bf16)
    wk_b = np.ascontiguousarray(wk_pad.reshape(DIM, H * DHP)).astype(bf16)
    wv_b = np.ascontiguousarray(wvT).astype(bf16)
    wp_b = np.ascontiguousarray(w_proj.T.reshape(H, DH, DIM)).astype(bf16)
    bias = np.ascontiguousarray(b_proj.reshape(DIM, 1))

    in_maps = []
    for c in range(NCORES):
        b, half = divmod(c, 2)
        xt = x[b].T  # [768, 2048]
        if half == 1:
            xt = np.concatenate([xt[:, NQ:], xt[:, :NQ]], axis=1)
        in_maps.append({
            "xt": np.ascontiguousarray(xt).astype(bf16),
            "wq": wq_b,
            "wk": wk_b,
            "wv": wv_b,
            "wp": wp_b,
            "bias": bias,
        })
    return in_maps


def kernel(x, w_qkv, w_proj, b_proj):
    from concourse.bass_utils import run_bass_kernel_spmd

    nc = _get_nc()
    in_maps = _prep_shards(x, w_qkv, w_proj, b_proj)
    res = run_bass_kernel_spmd(nc, in_maps, core_ids=list(range(NCORES)))
    out = np.empty((B, N, DIM), np.float32)
    for c in range(NCORES):
        b, half = divmod(c, 2)
        yT = np.asarray(res.results[c]["out"], dtype=np.float32)  # [768, 1024]
        out[b, half * NQ:(half + 1) * NQ, :] = yT.T
    return out


# revision 11
# speedup vs baseline: 1.4641x; 1.2623x over previous
"""Multi-head attention (B=4, N=2048, DIM=768, H=8, DH=96) on 8 TRN2 NeuronCores.

Sharding: data-parallel over (batch, query-half) — core c handles batch c//2,
query rows [(c%2)*1024, (c%2+1)*1024). Each core computes K/V for its full
batch (duplicated across the 2 cores sharing a batch), so there are NO
collectives: each core produces its own disjoint output shard.

Per-core compute (all matmuls bf16, fp32 PSUM accumulation):
  - Q^T/K^T projection in transposed space with head-dim padded 96->128 so
    each head's Q^T/K^T lands in its own 128-partition tiles.
  - V projection in natural space; a constant 1.0 column is appended per head
    (V|1) so the attn@V matmul also produces the softmax row-sums.
  - dots P^T[nk,nq]: lhsT=K^T[128dh, 128nk], rhs=Q^T[128dh, 512nq]; softmax
    scale folded into w_q host-side. exp() on ScalarE PSUM->SBUF(bf16), no
    max subtraction (logits max ~9, fp32-safe).
  - O'^T[97, nq] accumulated over 16 key tiles; row 96 = row-sum s.
  - normalize: evacuate O' to SBUF (frees PSUM fast), r=1/s via
    reciprocal_approx_fast, partition-broadcast r with a K=1 outer-product
    matmul, multiply.
  - proj: y^T[c,nq] accumulated over 8 heads in PSUM; bias added during
    eviction; DMA out.

Emission is software-pipelined: Q/K/V projection chunks are interleaved into
the attention t-loops so the ScalarE exp stream never starves and the PE
always has ready work.

Output per core: y^T [768, 1024] fp32; host transposes/reassembles.
"""

import numpy as np
import ml_dtypes

B, N, DIM = 4, 2048, 768
H, DH = 8, 96
DHP = 128          # padded head dim for Q/K tiles
NQ = N // 2        # query rows per core
SCALE = DH ** -0.5
NCORES = 8
CT = DIM // 128    # 6 contraction chunks
NT = N // 128      # 16 key tiles
NQC = NQ // 512    # 2 query chunks of 512
NKC = N // 512     # 4 key chunks of 512

_CACHE = {}


def _build():
    import concourse.mybir as mybir
    import concourse.tile as tile
    from concourse import bacc

    f32 = mybir.dt.float32
    bf16 = mybir.dt.bfloat16
    Exp = mybir.ActivationFunctionType.Exp
    mult = mybir.AluOpType.mult

    nc = bacc.Bacc("TRN2", debug=False, num_devices=NCORES)

    xt_d = nc.dram_tensor("xt", [DIM, N], bf16, kind="ExternalInput")
    wq_d = nc.dram_tensor("wq", [DIM, H * DHP], bf16, kind="ExternalInput")
    wk_d = nc.dram_tensor("wk", [DIM, H * DHP], bf16, kind="ExternalInput")
    wv_d = nc.dram_tensor("wv", [DIM, DIM], bf16, kind="ExternalInput")
    wp_d = nc.dram_tensor("wp", [H, DH, DIM], bf16, kind="ExternalInput")
    bias_d = nc.dram_tensor("bias", [DIM, 1], f32, kind="ExternalInput")
    out_d = nc.dram_tensor("out", [DIM, NQ], f32, kind="ExternalOutput")

    with tile.TileContext(nc) as tc:
        with (
            tc.tile_pool(name="const", bufs=1) as cpool,
            tc.tile_pool(name="ptp", bufs=4) as pt_pool,
            tc.tile_pool(name="onp", bufs=1) as on_pool,
            tc.tile_pool(name="smallp", bufs=2) as small_pool,
            tc.tile_pool(name="ysb", bufs=2) as y_pool,
            tc.tile_pool(name="ps_qkv", bufs=2, space="PSUM") as psum_qkv,
            tc.tile_pool(name="ps_d", bufs=3, space="PSUM") as psum_d,
            tc.tile_pool(name="ps_o", bufs=2, space="PSUM") as psum_o,
            tc.tile_pool(name="ps_rb", bufs=1, space="PSUM") as psum_rb,
        ):
            # ---- persistent SBUF tensors + input DMAs ----
            xt_sb = [cpool.tile([128, N], bf16, name=f"xt{t}") for t in range(CT)]
            wq_sb = [cpool.tile([128, H * DHP], bf16, name=f"wq{t}") for t in range(CT)]
            wk_sb = [cpool.tile([128, H * DHP], bf16, name=f"wk{t}") for t in range(CT)]
            wv_sb = [cpool.tile([128, DIM], bf16, name=f"wv{t}") for t in range(CT)]
            wp_sb = [cpool.tile([DH, DIM], bf16, name=f"wp{h}") for h in range(H)]
            bias_sb = [cpool.tile([128, 1], f32, name=f"bias{t}") for t in range(CT)]
            qt_sb = [
                [cpool.tile([DHP, 512], bf16, name=f"qt{h}_{qc}") for qc in range(NQC)]
                for h in range(H)
            ]
            kt_sb = [
                [cpool.tile([DHP, 512], bf16, name=f"kt{h}_{nc_}") for nc_ in range(NKC)]
                for h in range(H)
            ]
            v_sb = [cpool.tile([128, H, DH + 1], bf16, name=f"v{t}") for t in range(NT)]
            ones_sb = cpool.tile([128, DH], bf16, name="ones")

            for t in range(CT):
                nc.sync.dma_start(xt_sb[t][:], xt_d.ap()[t * 128:(t + 1) * 128, :])
            for t in range(CT):
                nc.sync.dma_start(wk_sb[t][:], wk_d.ap()[t * 128:(t + 1) * 128, :])
                nc.sync.dma_start(wq_sb[t][:], wq_d.ap()[t * 128:(t + 1) * 128, :])
                nc.sync.dma_start(wv_sb[t][:], wv_d.ap()[t * 128:(t + 1) * 128, :])
                nc.sync.dma_start(bias_sb[t][:], bias_d.ap()[t * 128:(t + 1) * 128, :])
            for h in range(H):
                nc.sync.dma_start(wp_sb[h][:], wp_d.ap()[h])

            nc.vector.memset(ones_sb[:], 1.0)
            for t in range(NT):
                nc.vector.memset(v_sb[t][:, :, DH:DH + 1], 1.0)

            # ---- chunk emitters ----
            def k_chunk(h, nc_):
                ps = psum_qkv.tile([128, 512], f32, name="kps", tag="qkvps")
                for ct in range(CT):
                    nc.tensor.matmul(
                        ps,
                        lhsT=wk_sb[ct][:, h * DHP:(h + 1) * DHP],
                        rhs=xt_sb[ct][:, nc_ * 512:(nc_ + 1) * 512],
                        start=(ct == 0),
                        stop=(ct == CT - 1),
                    )
                nc.vector.tensor_copy(out=kt_sb[h][nc_][:], in_=ps[:])

            def q_chunk(h, qc):
                ps = psum_qkv.tile([128, 512], f32, name="qps", tag="qkvps")
                for ct in range(CT):
                    nc.tensor.matmul(
                        ps,
                        lhsT=wq_sb[ct][:, h * DHP:(h + 1) * DHP],
                        rhs=xt_sb[ct][:, qc * 512:(qc + 1) * 512],
                        start=(ct == 0),
                        stop=(ct == CT - 1),
                    )
                nc.vector.tensor_copy(out=qt_sb[h][qc][:], in_=ps[:])

            def v_chunk(t, fc):
                ps = psum_qkv.tile([128, 512], f32, name="vps", tag="qkvps")
                vps = ps[:, :4 * DH]
                for ct in range(CT):
                    nc.tensor.matmul(
                        vps,
                        lhsT=xt_sb[ct][:, t * 128:(t + 1) * 128],
                        rhs=wv_sb[ct][:, fc * 4 * DH:(fc + 1) * 4 * DH],
                        start=(ct == 0),
                        stop=(ct == CT - 1),
                    )
                for j in range(4):
                    nc.vector.tensor_copy(
                        out=v_sb[t][:, fc * 4 + j, 0:DH],
                        in_=ps[:, j * DH:(j + 1) * DH],
                    )

            on_sb = [
                [on_pool.tile([DH, 512], bf16, name=f"on{h}_{qc}") for qc in range(NQC)]
                for h in range(H)
            ]

            # ---- attention for one head, with fillers interleaved per slot ----
            def attn_head(h, fillers):
                o_ps = [
                    psum_o.tile([DH + 1, 512], f32, name=f"ops{qc}", tag="ops")
                    for qc in range(NQC)
                ]
                for t in range(NT):
                    for qc in range(NQC):
                        d_ps = psum_d.tile([128, 512], f32, name="dps", tag="dps")
                        nc.tensor.matmul(
                            d_ps[:],
                            lhsT=kt_sb[h][t // 4][:, (t % 4) * 128:(t % 4 + 1) * 128],
                            rhs=qt_sb[h][qc][:],
                            start=True,
                            stop=True,
                        )
                        pt = pt_pool.tile([128, 512], bf16, name="pt", tag="pt")
                        nc.scalar.activation(pt[:], d_ps[:], Exp)
                        nc.tensor.matmul(
                            o_ps[qc],
                            lhsT=v_sb[t][:, h, :],
                            rhs=pt[:],
                            start=(t == 0),
                            stop=(t == NT - 1),
                        )
                    for fn in fillers.get(t, ()):
                        fn()
                # normalize: evacuate O' -> SBUF, r = 1/s, broadcast via K=1
                # outer product, multiply. All off the PE critical path.
                for qc in range(NQC):
                    o_st = small_pool.tile(
                        [DH + 1, 512], f32, name="ostage", tag="ostage", bufs=3
                    )
                    nc.vector.tensor_copy(out=o_st[:], in_=o_ps[qc][:])
                    s16 = small_pool.tile([DH + 1, 512], bf16, name="s16", tag="s16")
                    nc.vector.tensor_copy(out=s16[DH:DH + 1, :], in_=o_st[DH:DH + 1, :])
                    s0 = small_pool.tile([1, 512], bf16, name="s0", tag="s0")
                    nc.sync.dma_start(s0[:], s16[DH:DH + 1, :])
                    rbs = psum_rb.tile([DH, 512], f32, name="rbs", tag="rb")
                    nc.tensor.matmul(
                        rbs[:], lhsT=ones_sb[0:1, :], rhs=s0[:], start=True, stop=True
                    )
                    rinv = small_pool.tile([DH, 512], f32, name="rinv", tag="rinv")
                    nc.vector.reciprocal_approx_fast(out=rinv[:], in_=rbs[:])
                    nc.vector.tensor_tensor(
                        on_sb[h][qc][:], o_st[0:DH, :], rinv[:], mult
                    )

            # ---- software-pipelined emission ----
            # prologue: minimum to start attn(0): K chunk 0, both Q chunks,
            # V tiles 0-1 of the first head group.
            k_chunk(0, 0)
            q_chunk(0, 0)
            q_chunk(0, 1)
            v_chunk(0, 0)
            v_chunk(1, 0)

            # filler schedules per head
            def mk_fillers(h):
                f = {}

                def add(slot, fn):
                    f.setdefault(slot, []).append(fn)

                # remaining K chunks of this head, just-in-time
                for i, nc_ in enumerate((1, 2, 3)):
                    if h == 0:
                        add(4 * nc_ - 3, (lambda hh=h, n=nc_: k_chunk(hh, n)))
                    else:
                        add(2 * i, (lambda hh=h, n=nc_: k_chunk(hh, n)))
                # next head's first chunks late in this head
                if h + 1 < H:
                    add(10, lambda hh=h + 1: k_chunk(hh, 0))
                    add(12, lambda hh=h + 1: q_chunk(hh, 0))
                    add(14, lambda hh=h + 1: q_chunk(hh, 1))
                # V tiles: head 0 consumes fc=0 just-in-time; fc=1 spread over
                # heads 1-3.
                if h == 0:
                    for t in range(2, NT):
                        add(t - 2, lambda tt=t: v_chunk(tt, 0))
                elif h in (1, 2, 3):
                    start = [0, 6, 11][h - 1]
                    end = [6, 11, 16][h - 1]
                    slots = (1, 3, 5, 7, 9, 11)
                    for i, t in enumerate(range(start, end)):
                        add(slots[i], lambda tt=t: v_chunk(tt, 1))
                return f

            for h in range(H):
                attn_head(h, mk_fillers(h))

            # ---- output projection: y^T[c,nq] = sum_h wp[h]^T.T @ on[h] ----
            for ct in range(CT):
                y_sb = y_pool.tile([128, NQ], f32, name="y", tag="y")
                for qc in range(NQC):
                    yp = psum_d.tile([128, 512], f32, name="yps", tag="dps")
                    for h in range(H):
                        nc.tensor.matmul(
                            yp,
                            lhsT=wp_sb[h][:, ct * 128:(ct + 1) * 128],
                            rhs=on_sb[h][qc][:],
                            start=(h == 0),
                            stop=(h == H - 1),
                        )
                    nc.vector.tensor_scalar_add(
                        y_sb[:, qc * 512:(qc + 1) * 512], yp[:], bias_sb[ct][:]
                    )
                nc.sync.dma_start(out_d.ap()[ct * 128:(ct + 1) * 128, :], y_sb[:])

    nc.compile()
    return nc


def _get_nc():
    if "nc" not in _CACHE:
        _CACHE["nc"] = _build()
    return _CACHE["nc"]


def _prep_shards(x, w_qkv, w_proj, b_proj):
    bf16 = ml_dtypes.bfloat16
    x = np.asarray(x, dtype=np.float32)
    w_qkv = np.asarray(w_qkv, dtype=np.float32)
    w_proj = np.asarray(w_proj, dtype=np.float32)
    b_proj = np.asarray(b_proj, dtype=np.float32)

    # w_qkv: [3*INNER, DIM] rows: q rows [h*96+d], k rows 768+..., v rows 1536+...
    wqT = w_qkv[0:DIM].T.reshape(DIM, H, DH)        # [c, h, d]
    wkT = w_qkv[DIM:2 * DIM].T.reshape(DIM, H, DH)
    wvT = w_qkv[2 * DIM:3 * DIM].T                  # [c, f] natural head-major
    wq_pad = np.zeros((DIM, H, DHP), np.float32)
    wk_pad = np.zeros((DIM, H, DHP), np.float32)
    wq_pad[:, :, :DH] = wqT * SCALE
    wk_pad[:, :, :DH] = wkT
    wq_b = np.ascontiguousarray(wq_pad.reshape(DIM, H * DHP)).astype(bf16)
    wk_b = np.ascontiguousarray(wk_pad.reshape(DIM, H * DHP)).astype(bf16)
    wv_b = np.ascontiguousarray(wvT).astype(bf16)
    wp_b = np.ascontiguousarray(w_proj.T.reshape(H, DH, DIM)).astype(bf16)
    bias = np.ascontiguousarray(b_proj.reshape(DIM, 1))

    in_maps = []
    for c in range(NCORES):
        b, half = divmod(c, 2)
        xt = x[b].T  # [768, 2048]
        if half == 1:
            xt = np.concatenate([xt[:, NQ:], xt[:, :NQ]], axis=1)
        in_maps.append({
            "xt": np.ascontiguousarray(xt).astype(bf16),
            "wq": wq_b,
            "wk": wk_b,
            "wv": wv_b,
            "wp": wp_b,
            "bias": bias,
        })
    return in_maps


def kernel(x, w_qkv, w_proj, b_proj):
    from concourse.bass_utils import run_bass_kernel_spmd

    nc = _get_nc()
    in_maps = _prep_shards(x, w_qkv, w_proj, b_proj)
    res = run_bass_kernel_spmd(nc, in_maps, core_ids=list(range(NCORES)))
    out = np.empty((B, N, DIM), np.float32)
    for c in range(NCORES):
        b, half = divmod(c, 2)
        yT = np.asarray(res.results[c]["out"], dtype=np.float32)  # [768, 1024]
        out[b, half * NQ:(half + 1) * NQ, :] = yT.T
    return out
